# revision 1
# baseline (speedup 1.0000x reference)
"""GeneralAttention TRN2 Bass kernel.

reference:
    keys    = values @ W_attn.T                  [B, K, D]
    scores  = query @ keys^T                     [B, Q, K]
    attn    = softmax(scores, axis=-1)           [B, Q, K]
    context = attn @ values                      [B, Q, D]
    returns (context, attn)

B=16, Q=K=2048, D=512, fp32.  Sharded batch-parallel over 8 NeuronCores
(2 batches per core).  Matmul operands are cast to fp16 (full PE rate,
~1e-3 relative error on softmax weights); softmax statistics (max, exp,
sum, reciprocal) are fp32.  All on-chip transposes go through the DMA
xbar (16-bit path), so the tensor engine only runs the three matmul
stages.  Context rows are scaled by 1/rowsum after the matmul, which
lets the attn transpose consume the unnormalized exp(scores - max).
"""

import numpy as np

import concourse.bass as bass
import concourse.mybir as mybir
import concourse.tile as tile
from concourse import bacc, bass_utils

B, Q, K, D = 16, 2048, 2048, 512
NCORES = 8
BPC = B // NCORES  # batches per core

F32 = mybir.dt.float32
F16 = mybir.dt.float16
AF = mybir.ActivationFunctionType
ALU = mybir.AluOpType
AX = mybir.AxisListType
ts = bass.ts

DC = D // 128  # 4   d/e chunks of 128 (contraction tiles)
QT = Q // 128  # 16  q tiles
KB = K // 512  # 4   k blocks (one PSUM bank each)
KC = K // 128  # 16  k chunks

_CACHE = {}


def _build():
    nc = bacc.Bacc("TRN2", target_bir_lowering=False, debug=False,
                   num_devices=NCORES)

    q_d = nc.dram_tensor("query", [BPC, Q, D], F32, kind="ExternalInput").ap()
    v_d = nc.dram_tensor("values", [BPC, K, D], F32, kind="ExternalInput").ap()
    w_d = nc.dram_tensor("W_attn", [D, D], F32, kind="ExternalInput").ap()
    ctx_d = nc.dram_tensor("context", [BPC, Q, D], F32, kind="ExternalOutput").ap()
    att_d = nc.dram_tensor("attn", [BPC, Q, K], F32, kind="ExternalOutput").ap()

    with tile.TileContext(nc) as tc:
        with (
            tc.tile_pool(name="persist", bufs=1) as persist,
            tc.tile_pool(name="stage", bufs=3) as stage,
            tc.tile_pool(name="smx", bufs=3) as smx,
            tc.tile_pool(name="outp", bufs=3) as outp,
            tc.tile_pool(name="kp", bufs=2, space="PSUM") as kp_pool,
            tc.tile_pool(name="sc", bufs=4, space="PSUM") as sc_pool,
            tc.tile_pool(name="cp", bufs=2, space="PSUM") as cp_pool,
        ):
            # ---- W_attn prep (shared by both batches) ----
            # w16[e_p, e_chunk, d] = W[e_chunk*128 + e_p, d]
            w32 = stage.tile([128, DC, D], F32, tag="w32")
            for i in range(DC):
                nc.sync.dma_start(out=w32[:, i], in_=w_d[ts(i, 128)])
            w16 = persist.tile([128, DC, D], F16, tag="w16")
            nc.vector.tensor_copy(w16[:], w32[:])
            # wT16[d_p, d_chunk, e] = W[e, d_chunk*128 + d_p]
            wT16 = persist.tile([128, DC, D], F16, tag="wT16")
            for i in range(DC):
                nc.scalar.dma_start_transpose(
                    out=wT16[:, :, ts(i, 128)], in_=w16[:, i]
                )

            for b in range(BPC):
                # ---- load values; build v16 (natural) and vT16 ----
                # v16[k_p, k_chunk, d]  (context matmul rhs)
                v16 = persist.tile([128, KC, D], F16, tag="v16")
                # vT16[d_p, d_chunk, k] (keys matmul rhs)
                vT16 = persist.tile([128, DC, K], F16, tag="vT16")
                for t in range(KC):
                    v32 = stage.tile([128, D], F32, tag="v32")
                    nc.sync.dma_start(out=v32[:], in_=v_d[b, ts(t, 128)])
                    nc.vector.tensor_copy(v16[:, t], v32[:])
                    nc.scalar.dma_start_transpose(
                        out=vT16[:, :, ts(t, 128)], in_=v16[:, t]
                    )

                # ---- keysT16[e_p, e_chunk, k] = (values @ W.T)^T ----
                keysT16 = persist.tile([128, DC, K], F16, tag="keysT16")
                for i in range(DC):
                    for kb in range(KB):
                        kp = kp_pool.tile([128, 512], F32, tag="kp")
                        for j in range(DC):
                            nc.tensor.matmul(
                                kp[:],
                                wT16[:, j, ts(i, 128)],
                                vT16[:, j, ts(kb, 512)],
                                start=(j == 0),
                                stop=(j == DC - 1),
                            )
                        nc.scalar.copy(keysT16[:, i, ts(kb, 512)], kp[:])

                # ---- queryT16[e_p, e_chunk, q] ----
                qT16 = persist.tile([128, DC, Q], F16, tag="qT16")
                for t in range(QT):
                    q32 = stage.tile([128, D], F32, tag="q32")
                    nc.sync.dma_start(out=q32[:], in_=q_d[b, ts(t, 128)])
                    q16 = stage.tile([128, D], F16, tag="q16")
                    nc.vector.tensor_copy(q16[:], q32[:])
                    nc.scalar.dma_start_transpose(
                        out=qT16[:, :, ts(t, 128)], in_=q16[:]
                    )

                # ---- per q-tile: scores -> softmax -> context ----
                for qt in range(QT):
                    scs = []
                    mblk = smx.tile([128, KB], F32, tag="mblk")
                    for kb in range(KB):
                        sc = sc_pool.tile([128, 512], F32, tag="sc")
                        scs.append(sc)
                        for i in range(DC):
                            nc.tensor.matmul(
                                sc[:],
                                qT16[:, i, ts(qt, 128)],
                                keysT16[:, i, ts(kb, 512)],
                                start=(i == 0),
                                stop=(i == DC - 1),
                            )
                        nc.vector.tensor_reduce(
                            mblk[:, kb : kb + 1], sc[:], axis=AX.X, op=ALU.max
                        )
                    nm = smx.tile([128, 1], F32, tag="nm")
                    m = smx.tile([128, 1], F32, tag="m")
                    nc.vector.tensor_reduce(m[:], mblk[:], axis=AX.X, op=ALU.max)
                    nc.vector.tensor_scalar_mul(nm[:], m[:], -1.0)

                    # E16 = exp(scores - max) in fp16; row sums in fp32
                    E16 = smx.tile([128, K], F16, tag="E16")
                    ssub = smx.tile([128, KB], F32, tag="ssub")
                    for kb in range(KB):
                        nc.scalar.activation(
                            E16[:, ts(kb, 512)],
                            scs[kb][:],
                            AF.Exp,
                            bias=nm[:],
                            accum_out=ssub[:, kb : kb + 1],
                        )
                    s = smx.tile([128, 1], F32, tag="s")
                    nc.vector.tensor_reduce(s[:], ssub[:], axis=AX.X, op=ALU.add)
                    r = smx.tile([128, 1], F32, tag="r")
                    nc.vector.reciprocal(r[:], s[:])

                    # attn output (fp32) = E16 * r
                    attn32 = outp.tile([128, K], F32, tag="attn32")
                    nc.vector.tensor_scalar_mul(attn32[:], E16[:], r[:])
                    nc.sync.dma_start(out=att_d[b, ts(qt, 128)], in_=attn32[:])

                    # attnT[k_p, k_chunk, q] = E16[q, k]^T  (one xbar call)
                    aT16 = outp.tile([128, KC, 128], F16, tag="aT16")
                    nc.scalar.dma_start_transpose(out=aT16[:], in_=E16[:])

                    # context = (attnT.T @ v16) * r
                    cp = cp_pool.tile([128, D], F32, tag="cp")
                    for j in range(KC):
                        nc.tensor.matmul(
                            cp[:],
                            aT16[:, j],
                            v16[:, j],
                            start=(j == 0),
                            stop=(j == KC - 1),
                        )
                    ctx_sb = outp.tile([128, D], F32, tag="ctx_sb")
                    nc.vector.tensor_scalar_mul(ctx_sb[:], cp[:], r[:])
                    nc.sync.dma_start(out=ctx_d[b, ts(qt, 128)], in_=ctx_sb[:])

    nc.finalize()
    return nc


def kernel(query, values, W_attn):
    if "nc" not in _CACHE:
        _CACHE["nc"] = _build()
    nc = _CACHE["nc"]

    query = np.ascontiguousarray(query, dtype=np.float32)
    values = np.ascontiguousarray(values, dtype=np.float32)
    W_attn = np.ascontiguousarray(W_attn, dtype=np.float32)

    in_maps = []
    for c in range(NCORES):
        sl = slice(c * BPC, (c + 1) * BPC)
        in_maps.append(
            {"query": query[sl], "values": values[sl], "W_attn": W_attn}
        )

    res = bass_utils.run_bass_kernel_spmd(
        nc, in_maps, core_ids=list(range(NCORES))
    )
    context = np.concatenate([r["context"] for r in res.results], axis=0)
    attn = np.concatenate([r["attn"] for r in res.results], axis=0)
    return (context, attn)


# revision 4
# speedup vs baseline: 1.1041x; 1.1041x over previous
"""GeneralAttention TRN2 Bass kernel.

reference:
    keys    = values @ W_attn.T                  [B, K, D]
    scores  = query @ keys^T                     [B, Q, K]
    attn    = softmax(scores, axis=-1)           [B, Q, K]
    context = attn @ values                      [B, Q, D]
    returns (context, attn)

B=16, Q=K=2048, D=512, fp32.  Sharded batch-parallel over 8 NeuronCores
(2 batches per core).  Matmul operands are cast to fp16 (full PE rate,
~1e-3 relative error on softmax weights); softmax statistics (max, exp,
sum, reciprocal) are fp32.  All on-chip transposes go through the DMA
xbar (16-bit path), so the tensor engine only runs the three matmul
stages.  Context rows are scaled by 1/rowsum after the matmul, which
lets the attn transpose consume the unnormalized exp(scores - max).
"""

import numpy as np

import concourse.bass as bass
import concourse.mybir as mybir
import concourse.tile as tile
from concourse import bacc, bass_utils
from concourse.masks import make_identity

B, Q, K, D = 16, 2048, 2048, 512
NCORES = 8
BPC = B // NCORES  # batches per core

F32 = mybir.dt.float32
F16 = mybir.dt.float16
AF = mybir.ActivationFunctionType
ALU = mybir.AluOpType
AX = mybir.AxisListType
ts = bass.ts

DC = D // 128  # 4   d/e chunks of 128 (contraction tiles)
QT = Q // 128  # 16  q tiles
KB = K // 512  # 4   k blocks (one PSUM bank each)
KC = K // 128  # 16  k chunks

_CACHE = {}


def _build():
    nc = bacc.Bacc("TRN2", target_bir_lowering=False, debug=False,
                   num_devices=NCORES)

    q_d = nc.dram_tensor("query", [BPC, Q, D], F32, kind="ExternalInput").ap()
    v_d = nc.dram_tensor("values", [BPC, K, D], F32, kind="ExternalInput").ap()
    w_d = nc.dram_tensor("W_attn", [D, D], F32, kind="ExternalInput").ap()
    ctx_d = nc.dram_tensor("context", [BPC, Q, D], F32, kind="ExternalOutput").ap()
    att_d = nc.dram_tensor("attn", [BPC, Q, K], F32, kind="ExternalOutput").ap()

    with tile.TileContext(nc) as tc:
        with (
            tc.tile_pool(name="persist", bufs=1) as persist,
            tc.tile_pool(name="stage", bufs=2) as stage,
            tc.tile_pool(name="smx", bufs=3) as smx,
            tc.tile_pool(name="outp", bufs=2) as outp,
            tc.tile_pool(name="kp", bufs=2, space="PSUM") as kp_pool,
            tc.tile_pool(name="sc", bufs=4, space="PSUM") as sc_pool,
            tc.tile_pool(name="cp", bufs=2, space="PSUM") as cp_pool,
        ):
            # ---- W_attn prep (shared by both batches) ----
            # w16[e_p, e_chunk, d] = W[e_chunk*128 + e_p, d]
            w32 = stage.tile([128, DC, D], F32, tag="w32")
            for i in range(DC):
                nc.sync.dma_start(out=w32[:, i], in_=w_d[ts(i, 128)])
            w16 = persist.tile([128, DC, D], F16, tag="w16")
            nc.vector.tensor_copy(w16[:], w32[:])
            # wT16[d_p, d_chunk, e] = W[e, d_chunk*128 + d_p]
            wT16 = persist.tile([128, DC, D], F16, tag="wT16")
            for i in range(DC):
                nc.scalar.dma_start_transpose(
                    out=wT16[:, :, ts(i, 128)], in_=w16[:, i]
                )

            ident16 = persist.tile([128, 128], F16, tag="ident16")
            make_identity(nc, ident16[:])

            for b in range(BPC):
                # ---- load values; build v16 (natural) and vT16 ----
                # v16[k_p, k_chunk, d]  (context matmul rhs)
                v16 = persist.tile([128, KC, D], F16, tag="v16")
                # vT16[d_p, d_chunk, k] (keys matmul rhs)
                vT16 = persist.tile([128, DC, K], F16, tag="vT16")
                v_re = v_d[b].rearrange("(t p) d -> p t d", p=128)
                for g in range(KC // 4):
                    v32 = stage.tile([128, 4, D], F32, tag="v32")
                    nc.sync.dma_start(out=v32[:], in_=v_re[:, 4 * g : 4 * g + 4])
                    nc.vector.tensor_copy(v16[:, 4 * g : 4 * g + 4], v32[:])
                    for t in range(4 * g, 4 * g + 4):
                        nc.scalar.dma_start_transpose(
                            out=vT16[:, :, ts(t, 128)], in_=v16[:, t]
                        )

                # ---- keysT16[e_p, e_chunk, k] = (values @ W.T)^T ----
                keysT16 = persist.tile([128, DC, K], F16, tag="keysT16")
                for i in range(DC):
                    for kb in range(KB):
                        kp = kp_pool.tile([128, 512], F32, tag="kp")
                        for j in range(DC):
                            nc.tensor.matmul(
                                kp[:],
                                wT16[:, j, ts(i, 128)],
                                vT16[:, j, ts(kb, 512)],
                                start=(j == 0),
                                stop=(j == DC - 1),
                            )
                        nc.scalar.copy(keysT16[:, i, ts(kb, 512)], kp[:])

                # ---- queryT16[e_p, e_chunk, q] ----
                qT16 = persist.tile([128, DC, Q], F16, tag="qT16")
                q_re = q_d[b].rearrange("(t p) d -> p t d", p=128)
                for g in range(QT // 4):
                    q32 = stage.tile([128, 4, D], F32, tag="q32")
                    nc.sync.dma_start(out=q32[:], in_=q_re[:, 4 * g : 4 * g + 4])
                    q16 = stage.tile([128, 4, D], F16, tag="q16")
                    nc.vector.tensor_copy(q16[:], q32[:])
                    for t in range(4):
                        nc.scalar.dma_start_transpose(
                            out=qT16[:, :, ts(4 * g + t, 128)], in_=q16[:, t]
                        )

                # ---- per q-tile: scores -> softmax -> context ----
                for qt in range(QT):
                    scs = []
                    mblk = smx.tile([128, KB], F32, tag="mblk")
                    for kb in range(KB):
                        sc = sc_pool.tile([128, 512], F32, tag="sc")
                        scs.append(sc)
                        for i in range(DC):
                            nc.tensor.matmul(
                                sc[:],
                                qT16[:, i, ts(qt, 128)],
                                keysT16[:, i, ts(kb, 512)],
                                start=(i == 0),
                                stop=(i == DC - 1),
                            )
                        nc.vector.tensor_reduce(
                            mblk[:, kb : kb + 1], sc[:], axis=AX.X, op=ALU.max
                        )
                    nm = smx.tile([128, 1], F32, tag="nm")
                    m = smx.tile([128, 1], F32, tag="m")
                    nc.vector.tensor_reduce(m[:], mblk[:], axis=AX.X, op=ALU.max)
                    nc.vector.tensor_scalar_mul(nm[:], m[:], -1.0)

                    # E16 = exp(scores - max) in fp16; row sums in fp32
                    E16 = smx.tile([128, K], F16, tag="E16")
                    ssub = smx.tile([128, KB], F32, tag="ssub")
                    for kb in range(KB):
                        nc.scalar.activation(
                            E16[:, ts(kb, 512)],
                            scs[kb][:],
                            AF.Exp,
                            bias=nm[:],
                            accum_out=ssub[:, kb : kb + 1],
                        )
                    s = smx.tile([128, 1], F32, tag="s")
                    nc.vector.tensor_reduce(s[:], ssub[:], axis=AX.X, op=ALU.add)
                    r = smx.tile([128, 1], F32, tag="r")
                    nc.vector.reciprocal(r[:], s[:])

                    # attn output (fp32) = E16 * r
                    attn32 = outp.tile([128, K], F32, tag="attn32")
                    nc.vector.tensor_scalar_mul(attn32[:], E16[:], r[:])
                    nc.sync.dma_start(out=att_d[b, ts(qt, 128)], in_=attn32[:])

                    # attnT[k_p, k_chunk, q] = E16[q, k]^T via PE transpose
                    aT16 = outp.tile([128, KC, 128], F16, tag="aT16")
                    for j in range(KC):
                        tp = kp_pool.tile([128, 128], F16, tag="kp")
                        nc.tensor.transpose(
                            tp[:], E16[:, ts(j, 128)], ident16[:]
                        )
                        if j % 2:
                            nc.scalar.copy(aT16[:, j], tp[:])
                        else:
                            nc.vector.tensor_copy(aT16[:, j], tp[:])

                    # context = (attnT.T @ v16) * r
                    cp = cp_pool.tile([128, D], F32, tag="cp")
                    for j in range(KC):
                        nc.tensor.matmul(
                            cp[:],
                            aT16[:, j],
                            v16[:, j],
                            start=(j == 0),
                            stop=(j == KC - 1),
                        )
                    ctx_sb = outp.tile([128, D], F32, tag="ctx_sb")
                    nc.vector.tensor_scalar_mul(ctx_sb[:], cp[:], r[:])
                    nc.sync.dma_start(out=ctx_d[b, ts(qt, 128)], in_=ctx_sb[:])

    nc.finalize()
    return nc


def kernel(query, values, W_attn):
    if "nc" not in _CACHE:
        _CACHE["nc"] = _build()
    nc = _CACHE["nc"]

    query = np.ascontiguousarray(query, dtype=np.float32)
    values = np.ascontiguousarray(values, dtype=np.float32)
    W_attn = np.ascontiguousarray(W_attn, dtype=np.float32)

    in_maps = []
    for c in range(NCORES):
        sl = slice(c * BPC, (c + 1) * BPC)
        in_maps.append(
            {"query": query[sl], "values": values[sl], "W_attn": W_attn}
        )

    res = bass_utils.run_bass_kernel_spmd(
        nc, in_maps, core_ids=list(range(NCORES))
    )
    context = np.concatenate([r["context"] for r in res.results], axis=0)
    attn = np.concatenate([r["attn"] for r in res.results], axis=0)
    return (context, attn)
